# revision 50
# baseline (speedup 1.0000x reference)
"""Distributed causal multi-head attention for one TRN2 chip (8 NeuronCores).

Sharding: batch (2) x head-groups (4 heads/core) -> 8 cores.
Core c handles batch c//4, heads [ (c%4)*4 , (c%4)*4+4 ).
Per core: QKV projections for its 4 heads, flash-style causal attention
with scores kept transposed (S^T = K @ Q^T) so the PV product needs no
transposes; V is augmented with a ones column so the softmax denominators
fall out of the same matmul (row 64 of each head's O^T psum).  The output
projection is computed as a LOCAL PARTIAL product per core — att_local^T
(the core's 256 inner columns) against the matching 256-row slice of Wo,
giving a full-width [1024, seq] bf16 partial — and the host sums the four
partials of each batch group in fp32.  Same matmul cost as a gathered
projection, but no collectives at all: the TRN2 CC stream takes 20-50us
to initialize and its serial AllGathers run 4-28us each with huge
run-to-run variance, which otherwise dominates the kernel's tail.
Compute dtype bf16 (PSUM accumulation fp32), softmax in fp32.

Scheduling: the attention inner loop is software-pipelined one slot deep
(S^T of slot t+1 before PV of slot t) with a fine-grained work queue:
QKV-projection and output-projection matmuls are generators yielding one
matmul at a time, and two steps are popped between each slot's S and the
previous slot's PV, so the PE always has ~1us of independent work queued
while the exp of the previous slot completes on ACT.  The tile scheduler
reorders aggressively around its own timing model, so correctness of the
overlap rests on buffer depths (es bufs=8, s2 bufs=2) rather than
emission order.  The causal mask is applied as an identity-matmul
accumulation of an additive -30000 tile into the S psum (PE-local),
keeping DVE off the S->exp->PV critical path.  Work items carry
(deadline, earliest) slot positions: deadlines guarantee S/PV
dependencies are emitted in time on the in-order queues; projections of
chunk ci become eligible once ci's softmax normalization is done and the
last chunk's projections drain at the end (~5us tail).  Inputs arrive as
one wide DMA per weight (wq/wk pair-major halves so the first matmul
waits only on 256KB) and the x k-tiles load in chunk-column waves over
the three DMA-capable queues; only Q(pair0,chunk0) and the first K
j-tile are emitted inline so the first exp fires ~19us in.
"""

import sys
from collections import deque

import numpy as np

sys.path.insert(0, "/opt/trn_rl_repo")

import concourse.bass as bass  # noqa: E402
import concourse.bacc as bacc  # noqa: E402
import concourse.tile as tile  # noqa: E402
import concourse.mybir as mybir  # noqa: E402

F32 = mybir.dt.float32
BF16 = mybir.dt.bfloat16
ActFn = mybir.ActivationFunctionType

P = 128          # partition dim
CHUNK = 512      # i-chunk (matmul moving free dim, one psum bank of fp32)
DH = 64          # head dim
HPC = 4          # heads per core
HS = HPC * DH    # 256 per-core inner slice
DHA = DH + 1     # augmented head dim (ones column for softmax sums)
INNER = 1024     # total inner dim (16 heads x 64)
OUTD = 1024      # output dim (Wo columns)
N_CORES = 8
MASKNEG = -30000.0   # additive causal mask (exp(x-30000) == 0 in fp32)


def build_nc(seq=2048, dim=1024, n_cores=N_CORES, compile=True):
    """Build the SPMD Bass graph (identical on all cores)."""
    nch = seq // CHUNK          # i-chunks
    jpc = CHUNK // P            # j-tiles per chunk (4)
    njt = seq // P              # j-tiles
    nk = dim // P               # feature k-tiles
    nkl = HS // P               # local inner k-tiles for the partial proj
    nm = OUTD // P              # output m-blocks

    nc = bacc.Bacc("TRN2", target_bir_lowering=False, debug=False,
                   enable_asserts=False, num_devices=n_cores)

    xT = nc.dram_tensor("xT", [dim, seq], BF16, kind="ExternalInput").ap()
    # weights host-packed so each SBUF partition row is one contiguous DMA
    # row: [p, k*W+j] = W[k*128+p, j]
    wq = nc.dram_tensor("wq", [P, nk * HS], BF16, kind="ExternalInput").ap()
    wk = nc.dram_tensor("wk", [P, nk * HS], BF16, kind="ExternalInput").ap()
    wv = nc.dram_tensor("wv", [P, nk * HS], BF16, kind="ExternalInput").ap()
    # Wo ROW slice (this core's 256 inner rows), all 1024 columns
    wo = nc.dram_tensor("wo", [P, (HS // P) * OUTD], BF16,
                        kind="ExternalInput").ap()
    # [P, 0:2P] = additive causal mask duplicated for both heads,
    # [P, 2P:3P] = identity (mask accumulates into the S psum via matmul)
    mask_c = nc.dram_tensor("mask_c", [P, 3 * P], BF16,
                            kind="ExternalInput").ap()
    # full-width partial of the output projection; host sums the group
    outT = nc.dram_tensor("outT", [OUTD, seq], BF16,
                          kind="ExternalOutput").ap()

    with tile.TileContext(nc) as tc:
        with tc.tile_pool(name="sb", bufs=1) as sb, \
             tc.tile_pool(name="ps", bufs=1, space="PSUM") as ps:

            # ---- load inputs ----
            xt = [sb.tile([P, seq], BF16, tag=f"xt{k}", name=f"xt{k}")
                  for k in range(nk)]
            wq_sb = sb.tile([P, nk * HS], BF16, tag="wq", name="wq")
            wk_sb = sb.tile([P, nk * HS], BF16, tag="wk", name="wk")
            wv_sb = sb.tile([P, nk * HS], BF16, tag="wv", name="wv")
            wo_sb = sb.tile([P, nkl * OUTD], BF16, tag="wo", name="wo")
            mask_sb = sb.tile([P, 3 * P], BF16, tag="mask", name="mask")

            q3 = [nc.sync, nc.gpsimd, nc.scalar]
            qs = [nc.sync, nc.gpsimd]

            def ld_x(k, lo, hi, queue):
                if lo < hi:
                    queue.dma_start(xt[k][:, lo:hi],
                                    xT[k * P:(k + 1) * P, lo:hi])
            hw = nk * P
            # first transfers on each queue gate the first Q matmul; the
            # leading wq/wk slivers shrink the first matmuls' semaphore
            # deps to 64KB transfers instead of the full 256KB halves
            nc.scalar.dma_start(wq_sb[:, 0:2 * P], wq[:, 0:2 * P])
            for k in range(nk):
                ld_x(k, 0, CHUNK, q3[k % 3])
            nc.scalar.dma_start(wq_sb[:, 2 * P:hw], wq[:, 2 * P:hw])
            nc.scalar.dma_start(wk_sb[:, 0:2 * P], wk[:, 0:2 * P])
            nc.scalar.dma_start(wk_sb[:, 2 * P:hw], wk[:, 2 * P:hw])
            nc.gpsimd.dma_start(wv_sb[:], wv[:])
            nc.gpsimd.dma_start(mask_sb[:], mask_c[:])
            nc.scalar.dma_start(wq_sb[:, hw:2 * hw], wq[:, hw:2 * hw])
            nc.scalar.dma_start(wk_sb[:, hw:2 * hw], wk[:, hw:2 * hw])
            for k in range(nk):
                ld_x(k, CHUNK, min(2 * CHUNK, seq), qs[k % 2])
            nc.gpsimd.dma_start(wo_sb[:], wo[:])
            for k in range(nk):
                ld_x(k, 2 * CHUNK, seq, qs[k % 2])

            def wsl(w, k, a, b):
                return w[:, k * HS + a:k * HS + b]

            def wsl_pm(w, pair, k):
                # pair-major packed wq/wk: [p, (pair*nk + k)*128 + j]
                return w[:, (pair * nk + k) * P:(pair * nk + k + 1) * P]

            # persistent QKV results
            qt_sb = [sb.tile([P, seq], BF16, tag=f"qt{p}", name=f"qt{p}")
                     for p in range(2)]
            kt_sb = [sb.tile([P, seq], BF16, tag=f"kt{p}", name=f"kt{p}")
                     for p in range(2)]
            v_sb = [sb.tile([P, HPC * DHA], BF16, tag=f"v{j}", name=f"v{j}")
                    for j in range(njt)]
            ot_sb = [sb.tile([P, seq], BF16, tag=f"ot{p}", name=f"ot{p}")
                     for p in range(2)]
            # ones columns of the augmented V are set once up front
            for jt in range(njt):
                nc.vector.memset(
                    v_sb[jt].rearrange("p (h d) -> p h d", h=HPC)[:, :,
                                                                  DH:DHA],
                    1.0)

            # ---- work generators: one ~512-row matmul per yield ----
            def g_kt(pair, ch, lo=0, hi=CHUNK):
                pt = ps.tile([P, hi - lo], F32, tag="misc",
                             name=f"ktps{pair}_{ch}_{lo}", bufs=2)
                stride = 1 if hi - lo >= 384 else 2
                for k in range(nk):
                    nc.tensor.matmul(
                        pt[:], lhsT=wsl_pm(wk_sb, pair, k),
                        rhs=xt[k][:, ch * CHUNK + lo:ch * CHUNK + hi],
                        start=(k == 0), stop=(k == nk - 1))
                    if k % stride == stride - 1 and k < nk - 1:
                        yield
                nc.vector.tensor_copy(
                    kt_sb[pair][:, ch * CHUNK + lo:ch * CHUNK + hi], pt[:])
                yield

            def g_qt(pair, ch):
                pt = ps.tile([P, CHUNK], F32, tag="misc",
                             name=f"qps{pair}_{ch}", bufs=2)
                for k in range(nk):
                    nc.tensor.matmul(
                        pt[:], lhsT=wsl_pm(wq_sb, pair, k),
                        rhs=xt[k][:, ch * CHUNK:(ch + 1) * CHUNK],
                        start=(k == 0), stop=(k == nk - 1))
                    if k < nk - 1:
                        yield
                nc.vector.tensor_copy(
                    qt_sb[pair][:, ch * CHUNK:(ch + 1) * CHUNK], pt[:])
                yield

            def g_v(jt):
                # N=256 matmuls: two per yield to keep step granularity even
                pt = ps.tile([P, HS], F32, tag="misc",
                             name=f"vps{jt}", bufs=2)
                for k in range(nk):
                    nc.tensor.matmul(
                        pt[:], lhsT=xt[k][:, jt * P:(jt + 1) * P],
                        rhs=wsl(wv_sb, k, 0, HS),
                        start=(k == 0), stop=(k == nk - 1))
                    if k % 2 == 1 and k < nk - 1:
                        yield
                nc.vector.tensor_copy(
                    v_sb[jt].rearrange("p (h d) -> p h d", h=HPC)[:, :, 0:DH],
                    pt.rearrange("p (h d) -> p h d", h=HPC))
                yield

            def g_proj(ci, m):
                # local-partial output projection block: out_partial^T
                # [m*128:(m+1)*128, chunk ci] = Wo[local rows, m-block].T @
                # att_local^T[:, chunk] over the core's two inner k-tiles
                # (= the two head-pair slabs), straight from SBUF
                c0 = ci * CHUNK
                if ci == nch - 1 and m % 2 == 0:
                    # in the drain the s2 slot-pipeline banks are idle:
                    # spreading the projection psums over four buffers keeps
                    # each m-block from WAR-waiting on the previous evac
                    op_ps = ps.tile([P, 2 * CHUNK], F32, tag="s2",
                                    name=f"opz{ci}_{m}", bufs=2)[:, 0:CHUNK]
                else:
                    op_ps = ps.tile([P, CHUNK], F32, tag="misc",
                                    name=f"op{ci}_{m}", bufs=2)[:]
                for k in range(nkl):
                    nc.tensor.matmul(
                        op_ps,
                        lhsT=wo_sb[:,
                                   k * OUTD + m * P:k * OUTD + (m + 1) * P],
                        rhs=ot_sb[k][:, c0:c0 + CHUNK],
                        start=(k == 0), stop=(k == nkl - 1))
                    if k < nkl - 1:
                        yield
                o_sb = sb.tile([P, CHUNK], BF16, tag="osb",
                               name=f"o{ci}_{m}", bufs=3)
                nc.vector.tensor_copy(o_sb[:], op_ps)
                oq = nc.scalar if (ci == nch - 1 and m % 2 == 1) else nc.sync
                oq.dma_start(
                    outT[m * P:(m + 1) * P, c0:c0 + CHUNK], o_sb[:])
                yield

            # work:      [deadline, gen]  — QKV work, FIFO, deadline-forced
            # work_late: [earliest, deadline, gen] — projections, FIFO,
            #            popped once pos>=earliest (normalize done)
            work = deque()
            work_late = deque()
            INF = 1e9

            def drain_until(pos):
                while work and work[0][0] <= pos:
                    for _ in work[0][1]:
                        pass
                    work.popleft()
                while work_late and work_late[0][1] <= pos:
                    for _ in work_late[0][2]:
                        pass
                    work_late.popleft()

            def step_work(n, pos, slack=0.0):
                # pop n steps; keep popping (up to 2 extra) while the head's
                # deadline is within `slack` — spreads what would otherwise
                # be a forced burst of matmuls at a pass boundary
                popped = 0
                while popped < n + 2:
                    if work:
                        if popped >= n and work[0][0] > pos + slack:
                            break
                        try:
                            next(work[0][1])
                            popped += 1
                        except StopIteration:
                            work.popleft()
                    elif work_late and work_late[0][0] <= pos:
                        if popped >= n:
                            break
                        try:
                            next(work_late[0][2])
                            popped += 1
                        except StopIteration:
                            work_late.popleft()
                    else:
                        break

            # ---- upfront: only what slot 0's S needs is emitted inline
            for _ in g_qt(0, 0):
                pass
            for _ in g_kt(0, 0, 0, P):
                pass
            work.append([0.1, g_kt(0, 0, P, 2 * P)])
            work.append([0.12, g_v(0)])
            work.append([0.2, g_kt(0, 0, 2 * P, 3 * P)])
            work.append([0.22, g_v(1)])
            work.append([0.3, g_kt(0, 0, 3 * P, CHUNK)])
            work.append([0.32, g_v(2)])
            work.append([0.4, g_v(3)])
            work.append([0.42, g_qt(1, 0)])
            work.append([0.47, g_kt(1, 0, 0, 2 * P)])
            work.append([0.53, g_kt(1, 0, 2 * P, CHUNK)])

            # ---- attention chunks ----
            for ci in range(nch):
                jt_end = jpc * (ci + 1)
                c0 = ci * CHUNK

                if ci + 1 < nch:
                    njt_end = jpc * (ci + 2)
                    work.append([ci + 0.85, g_qt(0, ci + 1)])
                    work.append([ci + 0.90, g_kt(0, ci + 1, 0, 2 * P)])
                    work.append([ci + 0.96, g_kt(0, ci + 1, 2 * P, CHUNK)])
                    for jt in range(jpc * (ci + 1), jpc * (ci + 2)):
                        # consumed by PV(jt) in pass A of chunk ci+1
                        work.append(
                            [ci + 1 + jt / (2 * njt_end), g_v(jt)])
                    work.append([ci + 1.42, g_qt(1, ci + 1)])
                    work.append([ci + 1.46, g_kt(1, ci + 1, 0, 2 * P)])
                    work.append([ci + 1.52, g_kt(1, ci + 1, 2 * P, CHUNK)])

                ot_ps = {}
                pend = [None]

                def do_pass_end(hp, ci=ci, c0=c0, ot_ps=ot_ps):
                    # softmax normalize: rcp of each head's sum row (staged
                    # to SBUF — straight from PSUM it returns garbage on
                    # hardware), gpsimd broadcast, then one mul per head
                    # reading the O^T psum directly
                    for h2 in range(2):
                        srow = sb.tile([1, CHUNK], F32, tag=f"sr{h2}",
                                       name=f"sr{ci}_{hp}_{h2}", bufs=2)
                        nc.vector.tensor_copy(srow[:],
                                              ot_ps[hp][h2][DH:DHA, :])
                        rcp = sb.tile([1, CHUNK], F32, tag=f"rcp{h2}",
                                      name=f"rcp{ci}_{hp}_{h2}", bufs=2)
                        nc.vector.reciprocal_approx_fast(rcp[:], srow[:])
                        bc_sb = sb.tile([DH, CHUNK], F32, tag=f"bc{h2}",
                                        name=f"bc{ci}_{hp}_{h2}", bufs=2)
                        nc.gpsimd.partition_broadcast(bc_sb[:], rcp[:],
                                                      channels=DH)
                        nc.vector.tensor_mul(
                            ot_sb[hp][h2 * DH:(h2 + 1) * DH, c0:c0 + CHUNK],
                            ot_ps[hp][h2][0:DH, :],
                            bc_sb[:])

                def flush(jt_end=jt_end, ot_ps=ot_ps, pend=pend):
                    if pend[0] is None:
                        return
                    hp, jt, es, rel = pend[0]
                    pend[0] = None
                    for h2 in range(2):
                        h = 2 * hp + h2
                        nc.tensor.matmul(
                            ot_ps[hp][h2][:, rel:CHUNK],
                            lhsT=v_sb[jt][:, h * DHA:(h + 1) * DHA],
                            rhs=es[:, h2 * CHUNK + rel:(h2 + 1) * CHUNK],
                            start=(jt == 0), stop=(jt == jt_end - 1))
                    if jt == jt_end - 1:
                        do_pass_end(hp)

                for hp in range(2):
                    for jt in range(jt_end):
                        pos = ci + (hp * jt_end + jt) / (2 * jt_end)
                        drain_until(pos)
                        if jt == 0:
                            ot_ps[hp] = [
                                ps.tile([DHA, CHUNK], F32, tag=f"ot{h2}",
                                        name=f"ot{ci}_{hp}_{h2}", bufs=1)
                                for h2 in range(2)]
                        rel = max(0, (jt - jpc * ci)) * P
                        diag = jt >= jpc * ci

                        s2 = ps.tile([P, 2 * CHUNK], F32, tag="s2",
                                     name=f"s{ci}_{hp}_{jt}", bufs=2)
                        es = sb.tile([P, 2 * CHUNK], BF16, tag="es",
                                     name=f"es{ci}_{hp}_{jt}", bufs=10)

                        for h2 in range(2):
                            # S^T tile = K_h @ Q_h^T (row-tiled, K=64; the
                            # two heads run concurrently in the PE array)
                            nc.tensor.matmul(
                                s2[:, h2 * CHUNK + rel:(h2 + 1) * CHUNK],
                                lhsT=kt_sb[hp][h2 * DH:(h2 + 1) * DH,
                                               jt * P:(jt + 1) * P],
                                rhs=qt_sb[hp][h2 * DH:(h2 + 1) * DH,
                                              c0 + rel:c0 + CHUNK],
                                start=True, stop=not diag,
                                tile_position=(h2 * DH, 0))
                        if diag:
                            # additive causal mask accumulated into the S
                            # psum: identity.T @ maskneg tile (PE-local, no
                            # DVE in the S->exp->PV chain); one matmul per
                            # head — a single strided-AP matmul would cross
                            # the psum bank boundary between the two heads
                            for h2 in range(2):
                                nc.tensor.matmul(
                                    s2[:, h2 * CHUNK + rel:
                                       h2 * CHUNK + rel + P],
                                    lhsT=mask_sb[:, 2 * P:3 * P],
                                    rhs=mask_sb[:, h2 * P:(h2 + 1) * P],
                                    start=False, stop=True)
                        # one exp for both heads (both psum banks); flat AP
                        # off the diagonal (3D APs cost ~190ns extra on ACT)
                        if rel == 0:
                            nc.scalar.activation(es[:], s2[:], ActFn.Exp)
                        else:
                            nc.scalar.activation(
                                es.rearrange("p (t c) -> p t c",
                                             t=2)[:, :, rel:],
                                s2.rearrange("p (t c) -> p t c",
                                             t=2)[:, :, rel:],
                                ActFn.Exp)
                        # software pipeline: independent queued matmuls keep
                        # the PE streaming while exp(t) runs on ACT, then the
                        # previous slot's PV issues
                        # early chunks carry their own QKV supply in few
                        # slots: a deeper pop budget burns the backlog under
                        # the exps instead of in pass-boundary bursts
                        step_work(3 if ci < 2 else 2, pos,
                                  slack=2.5 / (2 * jt_end))
                        flush()
                        pend[0] = (hp, jt, es, rel)
                flush()

                # the chunk's partial projection needs only this chunk's
                # normalized ot_sb slabs — eligible shortly into the next
                # chunk, no communication involved
                for m in range(nm):
                    work_late.append([ci + 1.05, INF, g_proj(ci, m)])

            # final drain: leftover QKV steps, then the last chunk's
            # projections (~5us of PE work)
            while work:
                for _ in work[0][1]:
                    pass
                work.popleft()
            while work_late:
                for _ in work_late[0][2]:
                    pass
                work_late.popleft()

    if compile:
        nc.compile()
    return nc


def make_in_maps(x, Wq, Wk, Wv, Wo, n_cores=N_CORES):
    import ml_dtypes
    bf16 = ml_dtypes.bfloat16
    scale = np.float32(DH ** -0.5)
    # additive band mask for the diagonal j-tile of S^T [j,i]: 0 where
    # j <= i, -30000 where j > i (duplicated side by side for both heads),
    # plus the identity used to matmul-accumulate it into the S psum
    mask_b = np.where(np.triu(np.ones((P, P), np.float32)) > 0,
                      0.0, MASKNEG).astype(np.float32)
    ident = np.eye(P, dtype=np.float32)
    mask2 = np.concatenate([mask_b, mask_b, ident], axis=1).astype(bf16)

    def pack(sl):
        # [ntk*128, W] -> [128, ntk*W]: row p holds k-tile blocks side by
        # side so the whole weight is one contiguous-row DMA
        ntk = sl.shape[0] // P
        w = sl.shape[1]
        return np.ascontiguousarray(
            sl.reshape(ntk, P, w).transpose(1, 0, 2).reshape(P, ntk * w)
        ).astype(bf16)

    def pack_pm(sl):
        # pair-major: [nk*128, 2*128] -> [128, 2*nk*128] with
        # out[p, (pair*nk + k)*128 + j] = sl[k*128 + p, pair*128 + j],
        # so each head-pair's weights are one contiguous half
        ntk = sl.shape[0] // P
        return np.ascontiguousarray(
            sl.reshape(ntk, P, 2, P).transpose(1, 2, 0, 3).reshape(
                P, 2 * ntk * P)
        ).astype(bf16)

    in_maps = []
    for c in range(n_cores):
        b, r = divmod(c, 4)
        hs = r * HS
        in_maps.append({
            "xT": np.ascontiguousarray(x[b].T).astype(bf16),
            "wq": pack_pm(Wq[:, hs:hs + HS] * scale),
            "wk": pack_pm(Wk[:, hs:hs + HS]),
            "wv": pack(Wv[:, hs:hs + HS]),
            # row slice of Wo: this core's 256 inner rows, all columns
            "wo": pack(Wo[hs:hs + HS, :]),
            "mask_c": mask2,
        })
    return in_maps


def assemble_out(results, B, seq, n_cores=N_CORES):
    # sum the four per-core partials of each batch group in fp32
    out = np.zeros((B, seq, OUTD), np.float32)
    for c in range(n_cores):
        b, r = divmod(c, 4)
        out[b] += results[c]["outT"].T.astype(np.float32)
    return out


_NC_CACHE = {}


def kernel(x, Wq, Wk, Wv, Wo):
    from concourse import bass_utils
    x = np.asarray(x, np.float32)
    B, seq, dim = x.shape
    key = (seq, dim)
    if key not in _NC_CACHE:
        _NC_CACHE[key] = build_nc(seq=seq, dim=dim)
    nc = _NC_CACHE[key]
    in_maps = make_in_maps(x, np.asarray(Wq, np.float32),
                           np.asarray(Wk, np.float32),
                           np.asarray(Wv, np.float32),
                           np.asarray(Wo, np.float32))
    res = bass_utils.run_bass_kernel_spmd(
        nc, in_maps, core_ids=list(range(N_CORES)))
    return assemble_out(res.results, B, seq)


# revision 51
# speedup vs baseline: 1.0244x; 1.0244x over previous
"""Distributed causal multi-head attention for one TRN2 chip (8 NeuronCores).

Sharding: batch (2) x head-groups (4 heads/core) -> 8 cores.
Core c handles batch c//4, heads [ (c%4)*4 , (c%4)*4+4 ).
Per core: QKV projections for its 4 heads, flash-style causal attention
with scores kept transposed (S^T = K @ Q^T) so the PV product needs no
transposes; V is augmented with a ones column so the softmax denominators
fall out of the same matmul (row 64 of each head's O^T psum).  The output
projection is computed as a LOCAL PARTIAL product per core — att_local^T
(the core's 256 inner columns) against the matching 256-row slice of Wo,
giving a full-width [1024, seq] bf16 partial — and the host sums the four
partials of each batch group in fp32.  Same matmul cost as a gathered
projection, but no collectives at all: the TRN2 CC stream takes 20-50us
to initialize and its serial AllGathers run 4-28us each with huge
run-to-run variance, which otherwise dominates the kernel's tail.
Compute dtype bf16 (PSUM accumulation fp32), softmax in fp32.

Scheduling: the attention inner loop is software-pipelined one slot deep
(S^T of slot t+1 before PV of slot t) with a fine-grained work queue:
QKV-projection and output-projection matmuls are generators yielding one
matmul at a time, and two steps are popped between each slot's S and the
previous slot's PV, so the PE always has ~1us of independent work queued
while the exp of the previous slot completes on ACT.  The tile scheduler
reorders aggressively around its own timing model, so correctness of the
overlap rests on buffer depths (es bufs=8, s2 bufs=2) rather than
emission order.  The causal mask is applied as an identity-matmul
accumulation of an additive -30000 tile into the S psum (PE-local),
keeping DVE off the S->exp->PV critical path.  Work items carry
(deadline, earliest) slot positions: deadlines guarantee S/PV
dependencies are emitted in time on the in-order queues; projections of
chunk ci become eligible once ci's softmax normalization is done and the
last chunk's projections drain at the end (~5us tail).  Inputs arrive as
one wide DMA per weight (wq/wk pair-major halves so the first matmul
waits only on 256KB) and the x k-tiles load in chunk-column waves over
the three DMA-capable queues; only Q(pair0,chunk0) and the first K
j-tile are emitted inline so the first exp fires ~19us in.
"""

import sys
from collections import deque

import numpy as np

sys.path.insert(0, "/opt/trn_rl_repo")

import concourse.bass as bass  # noqa: E402
import concourse.bacc as bacc  # noqa: E402
import concourse.tile as tile  # noqa: E402
import concourse.mybir as mybir  # noqa: E402

F32 = mybir.dt.float32
BF16 = mybir.dt.bfloat16
ActFn = mybir.ActivationFunctionType

P = 128          # partition dim
CHUNK = 512      # i-chunk (matmul moving free dim, one psum bank of fp32)
DH = 64          # head dim
HPC = 4          # heads per core
HS = HPC * DH    # 256 per-core inner slice
DHA = DH + 1     # augmented head dim (ones column for softmax sums)
INNER = 1024     # total inner dim (16 heads x 64)
OUTD = 1024      # output dim (Wo columns)
N_CORES = 8
MASKNEG = -30000.0   # additive causal mask (exp(x-30000) == 0 in fp32)


def build_nc(seq=2048, dim=1024, n_cores=N_CORES, compile=True):
    """Build the SPMD Bass graph (identical on all cores)."""
    nch = seq // CHUNK          # i-chunks
    jpc = CHUNK // P            # j-tiles per chunk (4)
    njt = seq // P              # j-tiles
    nk = dim // P               # feature k-tiles
    nkl = HS // P               # local inner k-tiles for the partial proj
    nm = OUTD // P              # output m-blocks

    nc = bacc.Bacc("TRN2", target_bir_lowering=False, debug=False,
                   enable_asserts=False, num_devices=n_cores)

    xT = nc.dram_tensor("xT", [dim, seq], BF16, kind="ExternalInput").ap()
    # weights host-packed so each SBUF partition row is one contiguous DMA
    # row: [p, k*W+j] = W[k*128+p, j]
    wq = nc.dram_tensor("wq", [P, nk * HS], BF16, kind="ExternalInput").ap()
    wk = nc.dram_tensor("wk", [P, nk * HS], BF16, kind="ExternalInput").ap()
    wv = nc.dram_tensor("wv", [P, nk * HS], BF16, kind="ExternalInput").ap()
    # Wo ROW slice (this core's 256 inner rows), all 1024 columns
    wo = nc.dram_tensor("wo", [P, (HS // P) * OUTD], BF16,
                        kind="ExternalInput").ap()
    # [P, 0:2P] = additive causal mask duplicated for both heads,
    # [P, 2P:3P] = identity (mask accumulates into the S psum via matmul)
    mask_c = nc.dram_tensor("mask_c", [P, 3 * P], BF16,
                            kind="ExternalInput").ap()
    # full-width partial of the output projection; host sums the group
    outT = nc.dram_tensor("outT", [OUTD, seq], BF16,
                          kind="ExternalOutput").ap()

    with tile.TileContext(nc) as tc:
        with tc.tile_pool(name="sb", bufs=1) as sb, \
             tc.tile_pool(name="ps", bufs=1, space="PSUM") as ps:

            # ---- load inputs ----
            xt = [sb.tile([P, seq], BF16, tag=f"xt{k}", name=f"xt{k}")
                  for k in range(nk)]
            wq_sb = sb.tile([P, nk * HS], BF16, tag="wq", name="wq")
            wk_sb = sb.tile([P, nk * HS], BF16, tag="wk", name="wk")
            wv_sb = sb.tile([P, nk * HS], BF16, tag="wv", name="wv")
            wo_sb = sb.tile([P, nkl * OUTD], BF16, tag="wo", name="wo")
            mask_sb = sb.tile([P, 3 * P], BF16, tag="mask", name="mask")

            q3 = [nc.sync, nc.gpsimd, nc.scalar]
            qs = [nc.sync, nc.gpsimd]

            def ld_x(k, lo, hi, queue):
                if lo < hi:
                    queue.dma_start(xt[k][:, lo:hi],
                                    xT[k * P:(k + 1) * P, lo:hi])
            hw = nk * P
            # first transfers on each queue gate the first Q matmul
            nc.scalar.dma_start(wq_sb[:, 0:hw], wq[:, 0:hw])
            for k in range(nk):
                ld_x(k, 0, CHUNK, q3[k % 3])
            nc.scalar.dma_start(wk_sb[:, 0:hw], wk[:, 0:hw])
            nc.gpsimd.dma_start(wv_sb[:], wv[:])
            nc.gpsimd.dma_start(mask_sb[:], mask_c[:])
            nc.scalar.dma_start(wq_sb[:, hw:2 * hw], wq[:, hw:2 * hw])
            nc.scalar.dma_start(wk_sb[:, hw:2 * hw], wk[:, hw:2 * hw])
            for k in range(nk):
                ld_x(k, CHUNK, min(2 * CHUNK, seq), qs[k % 2])
            nc.gpsimd.dma_start(wo_sb[:], wo[:])
            for k in range(nk):
                ld_x(k, 2 * CHUNK, seq, qs[k % 2])

            def wsl(w, k, a, b):
                return w[:, k * HS + a:k * HS + b]

            def wsl_pm(w, pair, k):
                # pair-major packed wq/wk: [p, (pair*nk + k)*128 + j]
                return w[:, (pair * nk + k) * P:(pair * nk + k + 1) * P]

            # persistent QKV results
            qt_sb = [sb.tile([P, seq], BF16, tag=f"qt{p}", name=f"qt{p}")
                     for p in range(2)]
            kt_sb = [sb.tile([P, seq], BF16, tag=f"kt{p}", name=f"kt{p}")
                     for p in range(2)]
            v_sb = [sb.tile([P, HPC * DHA], BF16, tag=f"v{j}", name=f"v{j}")
                    for j in range(njt)]
            ot_sb = [sb.tile([P, seq], BF16, tag=f"ot{p}", name=f"ot{p}")
                     for p in range(2)]
            # ones columns of the augmented V are set once up front
            for jt in range(njt):
                nc.vector.memset(
                    v_sb[jt].rearrange("p (h d) -> p h d", h=HPC)[:, :,
                                                                  DH:DHA],
                    1.0)

            # ---- work generators: one ~512-row matmul per yield ----
            def g_kt(pair, ch, lo=0, hi=CHUNK):
                pt = ps.tile([P, hi - lo], F32, tag="misc",
                             name=f"ktps{pair}_{ch}_{lo}", bufs=2)
                stride = 1 if hi - lo >= 384 else 2
                for k in range(nk):
                    nc.tensor.matmul(
                        pt[:], lhsT=wsl_pm(wk_sb, pair, k),
                        rhs=xt[k][:, ch * CHUNK + lo:ch * CHUNK + hi],
                        start=(k == 0), stop=(k == nk - 1))
                    if k % stride == stride - 1 and k < nk - 1:
                        yield
                nc.vector.tensor_copy(
                    kt_sb[pair][:, ch * CHUNK + lo:ch * CHUNK + hi], pt[:])
                yield

            def g_qt(pair, ch):
                pt = ps.tile([P, CHUNK], F32, tag="misc",
                             name=f"qps{pair}_{ch}", bufs=2)
                for k in range(nk):
                    nc.tensor.matmul(
                        pt[:], lhsT=wsl_pm(wq_sb, pair, k),
                        rhs=xt[k][:, ch * CHUNK:(ch + 1) * CHUNK],
                        start=(k == 0), stop=(k == nk - 1))
                    if k < nk - 1:
                        yield
                nc.vector.tensor_copy(
                    qt_sb[pair][:, ch * CHUNK:(ch + 1) * CHUNK], pt[:])
                yield

            def g_v(jt):
                # N=256 matmuls: two per yield to keep step granularity even
                pt = ps.tile([P, HS], F32, tag="misc",
                             name=f"vps{jt}", bufs=2)
                for k in range(nk):
                    nc.tensor.matmul(
                        pt[:], lhsT=xt[k][:, jt * P:(jt + 1) * P],
                        rhs=wsl(wv_sb, k, 0, HS),
                        start=(k == 0), stop=(k == nk - 1))
                    if k % 2 == 1 and k < nk - 1:
                        yield
                nc.vector.tensor_copy(
                    v_sb[jt].rearrange("p (h d) -> p h d", h=HPC)[:, :, 0:DH],
                    pt.rearrange("p (h d) -> p h d", h=HPC))
                yield

            def g_proj(ci, m):
                # local-partial output projection block: out_partial^T
                # [m*128:(m+1)*128, chunk ci] = Wo[local rows, m-block].T @
                # att_local^T[:, chunk] over the core's two inner k-tiles
                # (= the two head-pair slabs), straight from SBUF
                c0 = ci * CHUNK
                if ci == nch - 1 and m % 2 == 0:
                    # in the drain the s2 slot-pipeline banks are idle:
                    # spreading the projection psums over four buffers keeps
                    # each m-block from WAR-waiting on the previous evac
                    op_ps = ps.tile([P, 2 * CHUNK], F32, tag="s2",
                                    name=f"opz{ci}_{m}", bufs=2)[:, 0:CHUNK]
                else:
                    op_ps = ps.tile([P, CHUNK], F32, tag="misc",
                                    name=f"op{ci}_{m}", bufs=2)[:]
                for k in range(nkl):
                    nc.tensor.matmul(
                        op_ps,
                        lhsT=wo_sb[:,
                                   k * OUTD + m * P:k * OUTD + (m + 1) * P],
                        rhs=ot_sb[k][:, c0:c0 + CHUNK],
                        start=(k == 0), stop=(k == nkl - 1))
                    if k < nkl - 1:
                        yield
                o_sb = sb.tile([P, CHUNK], BF16, tag="osb",
                               name=f"o{ci}_{m}", bufs=3)
                nc.vector.tensor_copy(o_sb[:], op_ps)
                oq = nc.scalar if (ci == nch - 1 and m % 2 == 1) else nc.sync
                oq.dma_start(
                    outT[m * P:(m + 1) * P, c0:c0 + CHUNK], o_sb[:])
                yield

            # work:      [deadline, gen]  — QKV work, FIFO, deadline-forced
            # work_late: [earliest, deadline, gen] — projections, FIFO,
            #            popped once pos>=earliest (normalize done)
            work = deque()
            work_late = deque()
            INF = 1e9

            def drain_until(pos):
                while work and work[0][0] <= pos:
                    for _ in work[0][1]:
                        pass
                    work.popleft()
                while work_late and work_late[0][1] <= pos:
                    for _ in work_late[0][2]:
                        pass
                    work_late.popleft()

            def step_work(n, pos, slack=0.0):
                # pop n steps; keep popping (up to 2 extra) while the head's
                # deadline is within `slack` — spreads what would otherwise
                # be a forced burst of matmuls at a pass boundary
                popped = 0
                while popped < n + 2:
                    if work:
                        if popped >= n and work[0][0] > pos + slack:
                            break
                        try:
                            next(work[0][1])
                            popped += 1
                        except StopIteration:
                            work.popleft()
                    elif work_late and work_late[0][0] <= pos:
                        if popped >= n:
                            break
                        try:
                            next(work_late[0][2])
                            popped += 1
                        except StopIteration:
                            work_late.popleft()
                    else:
                        break

            # ---- upfront: only what slot 0's S needs is emitted inline
            for _ in g_qt(0, 0):
                pass
            for _ in g_kt(0, 0, 0, P):
                pass
            work.append([0.1, g_kt(0, 0, P, 2 * P)])
            work.append([0.12, g_v(0)])
            work.append([0.2, g_kt(0, 0, 2 * P, 3 * P)])
            work.append([0.22, g_v(1)])
            work.append([0.3, g_kt(0, 0, 3 * P, CHUNK)])
            work.append([0.32, g_v(2)])
            work.append([0.4, g_v(3)])
            work.append([0.42, g_qt(1, 0)])
            work.append([0.47, g_kt(1, 0, 0, 2 * P)])
            work.append([0.53, g_kt(1, 0, 2 * P, CHUNK)])

            # ---- attention chunks ----
            for ci in range(nch):
                jt_end = jpc * (ci + 1)
                c0 = ci * CHUNK

                if ci + 1 < nch:
                    njt_end = jpc * (ci + 2)
                    work.append([ci + 0.85, g_qt(0, ci + 1)])
                    work.append([ci + 0.90, g_kt(0, ci + 1, 0, 2 * P)])
                    work.append([ci + 0.96, g_kt(0, ci + 1, 2 * P, CHUNK)])
                    for jt in range(jpc * (ci + 1), jpc * (ci + 2)):
                        # consumed by PV(jt) in pass A of chunk ci+1
                        work.append(
                            [ci + 1 + jt / (2 * njt_end), g_v(jt)])
                    work.append([ci + 1.42, g_qt(1, ci + 1)])
                    work.append([ci + 1.46, g_kt(1, ci + 1, 0, 2 * P)])
                    work.append([ci + 1.52, g_kt(1, ci + 1, 2 * P, CHUNK)])

                ot_ps = {}
                pend = [None]

                def do_pass_end(hp, ci=ci, c0=c0, ot_ps=ot_ps):
                    # softmax normalize: rcp of each head's sum row (staged
                    # to SBUF — straight from PSUM it returns garbage on
                    # hardware), gpsimd broadcast, then one mul per head
                    # reading the O^T psum directly
                    for h2 in range(2):
                        srow = sb.tile([1, CHUNK], F32, tag=f"sr{h2}",
                                       name=f"sr{ci}_{hp}_{h2}", bufs=2)
                        nc.vector.tensor_copy(srow[:],
                                              ot_ps[hp][h2][DH:DHA, :])
                        rcp = sb.tile([1, CHUNK], F32, tag=f"rcp{h2}",
                                      name=f"rcp{ci}_{hp}_{h2}", bufs=2)
                        nc.vector.reciprocal_approx_fast(rcp[:], srow[:])
                        bc_sb = sb.tile([DH, CHUNK], F32, tag=f"bc{h2}",
                                        name=f"bc{ci}_{hp}_{h2}", bufs=2)
                        nc.gpsimd.partition_broadcast(bc_sb[:], rcp[:],
                                                      channels=DH)
                        nc.vector.tensor_mul(
                            ot_sb[hp][h2 * DH:(h2 + 1) * DH, c0:c0 + CHUNK],
                            ot_ps[hp][h2][0:DH, :],
                            bc_sb[:])

                def flush(jt_end=jt_end, ot_ps=ot_ps, pend=pend):
                    if pend[0] is None:
                        return
                    hp, jt, es, rel = pend[0]
                    pend[0] = None
                    for h2 in range(2):
                        h = 2 * hp + h2
                        nc.tensor.matmul(
                            ot_ps[hp][h2][:, rel:CHUNK],
                            lhsT=v_sb[jt][:, h * DHA:(h + 1) * DHA],
                            rhs=es[:, h2 * CHUNK + rel:(h2 + 1) * CHUNK],
                            start=(jt == 0), stop=(jt == jt_end - 1))
                    if jt == jt_end - 1:
                        do_pass_end(hp)

                for hp in range(2):
                    for jt in range(jt_end):
                        pos = ci + (hp * jt_end + jt) / (2 * jt_end)
                        drain_until(pos)
                        if jt == 0:
                            ot_ps[hp] = [
                                ps.tile([DHA, CHUNK], F32, tag=f"ot{h2}",
                                        name=f"ot{ci}_{hp}_{h2}", bufs=1)
                                for h2 in range(2)]
                        rel = max(0, (jt - jpc * ci)) * P
                        diag = jt >= jpc * ci

                        s2 = ps.tile([P, 2 * CHUNK], F32, tag="s2",
                                     name=f"s{ci}_{hp}_{jt}", bufs=2)
                        es = sb.tile([P, 2 * CHUNK], BF16, tag="es",
                                     name=f"es{ci}_{hp}_{jt}", bufs=10)

                        for h2 in range(2):
                            # S^T tile = K_h @ Q_h^T (row-tiled, K=64; the
                            # two heads run concurrently in the PE array)
                            nc.tensor.matmul(
                                s2[:, h2 * CHUNK + rel:(h2 + 1) * CHUNK],
                                lhsT=kt_sb[hp][h2 * DH:(h2 + 1) * DH,
                                               jt * P:(jt + 1) * P],
                                rhs=qt_sb[hp][h2 * DH:(h2 + 1) * DH,
                                              c0 + rel:c0 + CHUNK],
                                start=True, stop=not diag,
                                tile_position=(h2 * DH, 0))
                        if diag:
                            # additive causal mask accumulated into the S
                            # psum: identity.T @ maskneg tile (PE-local, no
                            # DVE in the S->exp->PV chain); one matmul per
                            # head — a single strided-AP matmul would cross
                            # the psum bank boundary between the two heads
                            for h2 in range(2):
                                nc.tensor.matmul(
                                    s2[:, h2 * CHUNK + rel:
                                       h2 * CHUNK + rel + P],
                                    lhsT=mask_sb[:, 2 * P:3 * P],
                                    rhs=mask_sb[:, h2 * P:(h2 + 1) * P],
                                    start=False, stop=True)
                        # one exp for both heads (both psum banks); flat AP
                        # off the diagonal (3D APs cost ~190ns extra on ACT)
                        if rel == 0:
                            nc.scalar.activation(es[:], s2[:], ActFn.Exp)
                        else:
                            nc.scalar.activation(
                                es.rearrange("p (t c) -> p t c",
                                             t=2)[:, :, rel:],
                                s2.rearrange("p (t c) -> p t c",
                                             t=2)[:, :, rel:],
                                ActFn.Exp)
                        # software pipeline: independent queued matmuls keep
                        # the PE streaming while exp(t) runs on ACT, then the
                        # previous slot's PV issues
                        # early chunks carry their own QKV supply in few
                        # slots: a deeper pop budget burns the backlog under
                        # the exps instead of in pass-boundary bursts
                        step_work(3 if ci < 2 else 2, pos,
                                  slack=2.5 / (2 * jt_end))
                        flush()
                        pend[0] = (hp, jt, es, rel)
                flush()

                # the chunk's partial projection needs only this chunk's
                # normalized ot_sb slabs — eligible shortly into the next
                # chunk, no communication involved
                for m in range(nm):
                    work_late.append([ci + 1.05, INF, g_proj(ci, m)])

            # final drain: leftover QKV steps, then the last chunk's
            # projections (~5us of PE work)
            while work:
                for _ in work[0][1]:
                    pass
                work.popleft()
            while work_late:
                for _ in work_late[0][2]:
                    pass
                work_late.popleft()

    if compile:
        nc.compile()
    return nc


def make_in_maps(x, Wq, Wk, Wv, Wo, n_cores=N_CORES):
    import ml_dtypes
    bf16 = ml_dtypes.bfloat16
    scale = np.float32(DH ** -0.5)
    # additive band mask for the diagonal j-tile of S^T [j,i]: 0 where
    # j <= i, -30000 where j > i (duplicated side by side for both heads),
    # plus the identity used to matmul-accumulate it into the S psum
    mask_b = np.where(np.triu(np.ones((P, P), np.float32)) > 0,
                      0.0, MASKNEG).astype(np.float32)
    ident = np.eye(P, dtype=np.float32)
    mask2 = np.concatenate([mask_b, mask_b, ident], axis=1).astype(bf16)

    def pack(sl):
        # [ntk*128, W] -> [128, ntk*W]: row p holds k-tile blocks side by
        # side so the whole weight is one contiguous-row DMA
        ntk = sl.shape[0] // P
        w = sl.shape[1]
        return np.ascontiguousarray(
            sl.reshape(ntk, P, w).transpose(1, 0, 2).reshape(P, ntk * w)
        ).astype(bf16)

    def pack_pm(sl):
        # pair-major: [nk*128, 2*128] -> [128, 2*nk*128] with
        # out[p, (pair*nk + k)*128 + j] = sl[k*128 + p, pair*128 + j],
        # so each head-pair's weights are one contiguous half
        ntk = sl.shape[0] // P
        return np.ascontiguousarray(
            sl.reshape(ntk, P, 2, P).transpose(1, 2, 0, 3).reshape(
                P, 2 * ntk * P)
        ).astype(bf16)

    in_maps = []
    for c in range(n_cores):
        b, r = divmod(c, 4)
        hs = r * HS
        in_maps.append({
            "xT": np.ascontiguousarray(x[b].T).astype(bf16),
            "wq": pack_pm(Wq[:, hs:hs + HS] * scale),
            "wk": pack_pm(Wk[:, hs:hs + HS]),
            "wv": pack(Wv[:, hs:hs + HS]),
            # row slice of Wo: this core's 256 inner rows, all columns
            "wo": pack(Wo[hs:hs + HS, :]),
            "mask_c": mask2,
        })
    return in_maps


def assemble_out(results, B, seq, n_cores=N_CORES):
    # sum the four per-core partials of each batch group in fp32
    out = np.zeros((B, seq, OUTD), np.float32)
    for c in range(n_cores):
        b, r = divmod(c, 4)
        out[b] += results[c]["outT"].T.astype(np.float32)
    return out


_NC_CACHE = {}


def kernel(x, Wq, Wk, Wv, Wo):
    from concourse import bass_utils
    x = np.asarray(x, np.float32)
    B, seq, dim = x.shape
    key = (seq, dim)
    if key not in _NC_CACHE:
        _NC_CACHE[key] = build_nc(seq=seq, dim=dim)
    nc = _NC_CACHE[key]
    in_maps = make_in_maps(x, np.asarray(Wq, np.float32),
                           np.asarray(Wk, np.float32),
                           np.asarray(Wv, np.float32),
                           np.asarray(Wo, np.float32))
    res = bass_utils.run_bass_kernel_spmd(
        nc, in_maps, core_ids=list(range(N_CORES)))
    return assemble_out(res.results, B, seq)


# revision 52
# speedup vs baseline: 1.0310x; 1.0064x over previous
"""Distributed causal multi-head attention for one TRN2 chip (8 NeuronCores).

Sharding: batch (2) x head-groups (4 heads/core) -> 8 cores.
Core c handles batch c//4, heads [ (c%4)*4 , (c%4)*4+4 ).
Per core: QKV projections for its 4 heads, flash-style causal attention
with scores kept transposed (S^T = K @ Q^T) so the PV product needs no
transposes; V is augmented with a ones column so the softmax denominators
fall out of the same matmul (row 64 of each head's O^T psum).  The output
projection is computed as a LOCAL PARTIAL product per core — att_local^T
(the core's 256 inner columns) against the matching 256-row slice of Wo,
giving a full-width [1024, seq] bf16 partial — and the host sums the four
partials of each batch group in fp32.  Same matmul cost as a gathered
projection, but no collectives at all: the TRN2 CC stream takes 20-50us
to initialize and its serial AllGathers run 4-28us each with huge
run-to-run variance, which otherwise dominates the kernel's tail.
Compute dtype bf16 (PSUM accumulation fp32), softmax in fp32.

Scheduling: the attention inner loop is software-pipelined one slot deep
(S^T of slot t+1 before PV of slot t) with a fine-grained work queue:
QKV-projection and output-projection matmuls are generators yielding one
matmul at a time, and two steps are popped between each slot's S and the
previous slot's PV, so the PE always has ~1us of independent work queued
while the exp of the previous slot completes on ACT.  The tile scheduler
reorders aggressively around its own timing model, so correctness of the
overlap rests on buffer depths (es bufs=8, s2 bufs=2) rather than
emission order.  The causal mask is applied as an identity-matmul
accumulation of an additive -30000 tile into the S psum (PE-local),
keeping DVE off the S->exp->PV critical path.  Work items carry
(deadline, earliest) slot positions: deadlines guarantee S/PV
dependencies are emitted in time on the in-order queues; projections of
chunk ci become eligible once ci's softmax normalization is done and the
last chunk's projections drain at the end (~5us tail).  Inputs arrive as
one wide DMA per weight (wq/wk pair-major halves so the first matmul
waits only on 256KB) and the x k-tiles load in chunk-column waves over
the three DMA-capable queues; only Q(pair0,chunk0) and the first K
j-tile are emitted inline so the first exp fires ~19us in.
"""

import sys
from collections import deque

import numpy as np

sys.path.insert(0, "/opt/trn_rl_repo")

import concourse.bass as bass  # noqa: E402
import concourse.bacc as bacc  # noqa: E402
import concourse.tile as tile  # noqa: E402
import concourse.mybir as mybir  # noqa: E402

F32 = mybir.dt.float32
BF16 = mybir.dt.bfloat16
ActFn = mybir.ActivationFunctionType

P = 128          # partition dim
CHUNK = 512      # i-chunk (matmul moving free dim, one psum bank of fp32)
DH = 64          # head dim
HPC = 4          # heads per core
HS = HPC * DH    # 256 per-core inner slice
DHA = DH + 1     # augmented head dim (ones column for softmax sums)
INNER = 1024     # total inner dim (16 heads x 64)
OUTD = 1024      # output dim (Wo columns)
N_CORES = 8
MASKNEG = -30000.0   # additive causal mask (exp(x-30000) == 0 in fp32)


def build_nc(seq=2048, dim=1024, n_cores=N_CORES, compile=True):
    """Build the SPMD Bass graph (identical on all cores)."""
    nch = seq // CHUNK          # i-chunks
    jpc = CHUNK // P            # j-tiles per chunk (4)
    njt = seq // P              # j-tiles
    nk = dim // P               # feature k-tiles
    nkl = HS // P               # local inner k-tiles for the partial proj
    nm = OUTD // P              # output m-blocks

    nc = bacc.Bacc("TRN2", target_bir_lowering=False, debug=False,
                   enable_asserts=False, num_devices=n_cores)

    xT = nc.dram_tensor("xT", [dim, seq], BF16, kind="ExternalInput").ap()
    # weights host-packed so each SBUF partition row is one contiguous DMA
    # row: [p, k*W+j] = W[k*128+p, j]
    wq = nc.dram_tensor("wq", [P, nk * HS], BF16, kind="ExternalInput").ap()
    wk = nc.dram_tensor("wk", [P, nk * HS], BF16, kind="ExternalInput").ap()
    wv = nc.dram_tensor("wv", [P, nk * HS], BF16, kind="ExternalInput").ap()
    # Wo ROW slice (this core's 256 inner rows), all 1024 columns
    wo = nc.dram_tensor("wo", [P, (HS // P) * OUTD], BF16,
                        kind="ExternalInput").ap()
    # [P, 0:2P] = additive causal mask duplicated for both heads,
    # [P, 2P:3P] = identity (mask accumulates into the S psum via matmul)
    mask_c = nc.dram_tensor("mask_c", [P, 3 * P], BF16,
                            kind="ExternalInput").ap()
    # full-width partial of the output projection; host sums the group
    outT = nc.dram_tensor("outT", [OUTD, seq], BF16,
                          kind="ExternalOutput").ap()

    with tile.TileContext(nc) as tc:
        with tc.tile_pool(name="sb", bufs=1) as sb, \
             tc.tile_pool(name="ps", bufs=1, space="PSUM") as ps:

            # ---- load inputs ----
            xt = [sb.tile([P, seq], BF16, tag=f"xt{k}", name=f"xt{k}")
                  for k in range(nk)]
            wq_sb = sb.tile([P, nk * HS], BF16, tag="wq", name="wq")
            wk_sb = sb.tile([P, nk * HS], BF16, tag="wk", name="wk")
            wv_sb = sb.tile([P, nk * HS], BF16, tag="wv", name="wv")
            wo_sb = sb.tile([P, nkl * OUTD], BF16, tag="wo", name="wo")
            mask_sb = sb.tile([P, 3 * P], BF16, tag="mask", name="mask")

            q3 = [nc.sync, nc.gpsimd, nc.scalar]
            qs = [nc.sync, nc.gpsimd]

            def ld_x(k, lo, hi, queue):
                if lo < hi:
                    queue.dma_start(xt[k][:, lo:hi],
                                    xT[k * P:(k + 1) * P, lo:hi])
            hw = nk * P
            # first transfers on each queue gate the first Q matmul
            nc.scalar.dma_start(wq_sb[:, 0:hw], wq[:, 0:hw])
            for k in range(nk):
                ld_x(k, 0, CHUNK, q3[k % 3])
            nc.scalar.dma_start(wk_sb[:, 0:hw], wk[:, 0:hw])
            nc.gpsimd.dma_start(wv_sb[:], wv[:])
            nc.gpsimd.dma_start(mask_sb[:], mask_c[:])
            nc.scalar.dma_start(wq_sb[:, hw:2 * hw], wq[:, hw:2 * hw])
            nc.scalar.dma_start(wk_sb[:, hw:2 * hw], wk[:, hw:2 * hw])
            for k in range(nk):
                ld_x(k, CHUNK, min(2 * CHUNK, seq), qs[k % 2])
            nc.gpsimd.dma_start(wo_sb[:], wo[:])
            for k in range(nk):
                ld_x(k, 2 * CHUNK, seq, qs[k % 2])

            def wsl(w, k, a, b):
                return w[:, k * HS + a:k * HS + b]

            def wsl_pm(w, pair, k):
                # pair-major packed wq/wk: [p, (pair*nk + k)*128 + j]
                return w[:, (pair * nk + k) * P:(pair * nk + k + 1) * P]

            # persistent QKV results
            qt_sb = [sb.tile([P, seq], BF16, tag=f"qt{p}", name=f"qt{p}")
                     for p in range(2)]
            kt_sb = [sb.tile([P, seq], BF16, tag=f"kt{p}", name=f"kt{p}")
                     for p in range(2)]
            v_sb = [sb.tile([P, HPC * DHA], BF16, tag=f"v{j}", name=f"v{j}")
                    for j in range(njt)]
            ot_sb = [sb.tile([P, seq], BF16, tag=f"ot{p}", name=f"ot{p}")
                     for p in range(2)]
            # ones columns of the augmented V are set once up front
            for jt in range(njt):
                nc.vector.memset(
                    v_sb[jt].rearrange("p (h d) -> p h d", h=HPC)[:, :,
                                                                  DH:DHA],
                    1.0)

            # ---- work generators: one ~512-row matmul per yield ----
            def g_kt(pair, ch, lo=0, hi=CHUNK):
                pt = ps.tile([P, hi - lo], F32, tag="misc",
                             name=f"ktps{pair}_{ch}_{lo}", bufs=2)
                stride = 1 if hi - lo >= 384 else 2
                for k in range(nk):
                    nc.tensor.matmul(
                        pt[:], lhsT=wsl_pm(wk_sb, pair, k),
                        rhs=xt[k][:, ch * CHUNK + lo:ch * CHUNK + hi],
                        start=(k == 0), stop=(k == nk - 1))
                    if k % stride == stride - 1 and k < nk - 1:
                        yield
                nc.vector.tensor_copy(
                    kt_sb[pair][:, ch * CHUNK + lo:ch * CHUNK + hi], pt[:])
                yield

            def g_qt(pair, ch):
                pt = ps.tile([P, CHUNK], F32, tag="misc",
                             name=f"qps{pair}_{ch}", bufs=2)
                for k in range(nk):
                    nc.tensor.matmul(
                        pt[:], lhsT=wsl_pm(wq_sb, pair, k),
                        rhs=xt[k][:, ch * CHUNK:(ch + 1) * CHUNK],
                        start=(k == 0), stop=(k == nk - 1))
                    if k < nk - 1:
                        yield
                nc.vector.tensor_copy(
                    qt_sb[pair][:, ch * CHUNK:(ch + 1) * CHUNK], pt[:])
                yield

            def g_v(jt):
                # N=256 matmuls: two per yield to keep step granularity even
                pt = ps.tile([P, HS], F32, tag="misc",
                             name=f"vps{jt}", bufs=2)
                for k in range(nk):
                    nc.tensor.matmul(
                        pt[:], lhsT=xt[k][:, jt * P:(jt + 1) * P],
                        rhs=wsl(wv_sb, k, 0, HS),
                        start=(k == 0), stop=(k == nk - 1))
                    if k % 2 == 1 and k < nk - 1:
                        yield
                nc.vector.tensor_copy(
                    v_sb[jt].rearrange("p (h d) -> p h d", h=HPC)[:, :, 0:DH],
                    pt.rearrange("p (h d) -> p h d", h=HPC))
                yield

            def g_proj(ci, m):
                # local-partial output projection block: out_partial^T
                # [m*128:(m+1)*128, chunk ci] = Wo[local rows, m-block].T @
                # att_local^T[:, chunk] over the core's two inner k-tiles
                # (= the two head-pair slabs), straight from SBUF
                c0 = ci * CHUNK
                if ci == nch - 1 and m % 2 == 0:
                    # in the drain the s2 slot-pipeline banks are idle:
                    # spreading the projection psums over four buffers keeps
                    # each m-block from WAR-waiting on the previous evac
                    op_ps = ps.tile([P, 2 * CHUNK], F32, tag="s2",
                                    name=f"opz{ci}_{m}", bufs=2)[:, 0:CHUNK]
                else:
                    op_ps = ps.tile([P, CHUNK], F32, tag="misc",
                                    name=f"op{ci}_{m}", bufs=2)[:]
                for k in range(nkl):
                    nc.tensor.matmul(
                        op_ps,
                        lhsT=wo_sb[:,
                                   k * OUTD + m * P:k * OUTD + (m + 1) * P],
                        rhs=ot_sb[k][:, c0:c0 + CHUNK],
                        start=(k == 0), stop=(k == nkl - 1))
                    if k < nkl - 1:
                        yield
                o_sb = sb.tile([P, CHUNK], BF16, tag="osb",
                               name=f"o{ci}_{m}", bufs=3)
                nc.vector.tensor_copy(o_sb[:], op_ps)
                oq = nc.scalar if (ci == nch - 1 and m % 2 == 1) else nc.sync
                oq.dma_start(
                    outT[m * P:(m + 1) * P, c0:c0 + CHUNK], o_sb[:])
                yield

            # work:      [deadline, gen]  — QKV work, FIFO, deadline-forced
            # work_late: [earliest, deadline, gen] — projections, FIFO,
            #            popped once pos>=earliest (normalize done)
            work = deque()
            work_late = deque()
            INF = 1e9

            def drain_until(pos):
                while work and work[0][0] <= pos:
                    for _ in work[0][1]:
                        pass
                    work.popleft()
                while work_late and work_late[0][1] <= pos:
                    for _ in work_late[0][2]:
                        pass
                    work_late.popleft()

            def step_work(n, pos, slack=0.0):
                # pop n steps; keep popping (up to 2 extra) while the head's
                # deadline is within `slack` — spreads what would otherwise
                # be a forced burst of matmuls at a pass boundary
                popped = 0
                while popped < n + 2:
                    if work:
                        if popped >= n and work[0][0] > pos + slack:
                            break
                        try:
                            next(work[0][1])
                            popped += 1
                        except StopIteration:
                            work.popleft()
                    elif work_late and work_late[0][0] <= pos:
                        if popped >= n:
                            break
                        try:
                            next(work_late[0][2])
                            popped += 1
                        except StopIteration:
                            work_late.popleft()
                    else:
                        break

            # ---- upfront: only what slot 0's S needs is emitted inline
            for _ in g_qt(0, 0):
                pass
            for _ in g_kt(0, 0, 0, P):
                pass
            work.append([0.1, g_kt(0, 0, P, 2 * P)])
            work.append([0.12, g_v(0)])
            work.append([0.2, g_kt(0, 0, 2 * P, 3 * P)])
            work.append([0.22, g_v(1)])
            work.append([0.3, g_kt(0, 0, 3 * P, CHUNK)])
            work.append([0.32, g_v(2)])
            work.append([0.4, g_v(3)])
            work.append([0.42, g_qt(1, 0)])
            work.append([0.47, g_kt(1, 0, 0, 2 * P)])
            work.append([0.53, g_kt(1, 0, 2 * P, CHUNK)])

            # ---- attention chunks ----
            for ci in range(nch):
                jt_end = jpc * (ci + 1)
                c0 = ci * CHUNK

                if ci + 1 < nch:
                    njt_end = jpc * (ci + 2)
                    work.append([ci + 0.85, g_qt(0, ci + 1)])
                    work.append([ci + 0.90, g_kt(0, ci + 1, 0, 2 * P)])
                    work.append([ci + 0.96, g_kt(0, ci + 1, 2 * P, CHUNK)])
                    for jt in range(jpc * (ci + 1), jpc * (ci + 2)):
                        # consumed by PV(jt) in pass A of chunk ci+1
                        work.append(
                            [ci + 1 + jt / (2 * njt_end), g_v(jt)])
                    work.append([ci + 1.42, g_qt(1, ci + 1)])
                    work.append([ci + 1.46, g_kt(1, ci + 1, 0, 2 * P)])
                    work.append([ci + 1.52, g_kt(1, ci + 1, 2 * P, CHUNK)])

                ot_ps = {}
                pend = [None]

                def do_pass_end(hp, ci=ci, c0=c0, ot_ps=ot_ps):
                    # softmax normalize: rcp of each head's sum row (staged
                    # to SBUF — straight from PSUM it returns garbage on
                    # hardware), gpsimd broadcast, then one mul per head
                    # reading the O^T psum directly
                    for h2 in range(2):
                        srow = sb.tile([1, CHUNK], F32, tag=f"sr{h2}",
                                       name=f"sr{ci}_{hp}_{h2}", bufs=2)
                        nc.vector.tensor_copy(srow[:],
                                              ot_ps[hp][h2][DH:DHA, :])
                        rcp = sb.tile([1, CHUNK], F32, tag=f"rcp{h2}",
                                      name=f"rcp{ci}_{hp}_{h2}", bufs=2)
                        nc.vector.reciprocal_approx_fast(rcp[:], srow[:])
                        bc_sb = sb.tile([DH, CHUNK], F32, tag=f"bc{h2}",
                                        name=f"bc{ci}_{hp}_{h2}", bufs=2)
                        nc.gpsimd.partition_broadcast(bc_sb[:], rcp[:],
                                                      channels=DH)
                        nc.vector.tensor_mul(
                            ot_sb[hp][h2 * DH:(h2 + 1) * DH, c0:c0 + CHUNK],
                            ot_ps[hp][h2][0:DH, :],
                            bc_sb[:])

                def flush(jt_end=jt_end, ot_ps=ot_ps, pend=pend):
                    if pend[0] is None:
                        return
                    hp, jt, es, rel = pend[0]
                    pend[0] = None
                    for h2 in range(2):
                        h = 2 * hp + h2
                        nc.tensor.matmul(
                            ot_ps[hp][h2][:, rel:CHUNK],
                            lhsT=v_sb[jt][:, h * DHA:(h + 1) * DHA],
                            rhs=es[:, h2 * CHUNK + rel:(h2 + 1) * CHUNK],
                            start=(jt == 0), stop=(jt == jt_end - 1))
                    if jt == jt_end - 1:
                        do_pass_end(hp)

                for hp in range(2):
                    for jt in range(jt_end):
                        pos = ci + (hp * jt_end + jt) / (2 * jt_end)
                        drain_until(pos)
                        if jt == 0:
                            ot_ps[hp] = [
                                ps.tile([DHA, CHUNK], F32, tag=f"ot{h2}",
                                        name=f"ot{ci}_{hp}_{h2}", bufs=1)
                                for h2 in range(2)]
                        rel = max(0, (jt - jpc * ci)) * P
                        diag = jt >= jpc * ci

                        s2 = ps.tile([P, 2 * CHUNK], F32, tag="s2",
                                     name=f"s{ci}_{hp}_{jt}", bufs=2)
                        es = sb.tile([P, 2 * CHUNK], BF16, tag="es",
                                     name=f"es{ci}_{hp}_{jt}", bufs=12)

                        for h2 in range(2):
                            # S^T tile = K_h @ Q_h^T (row-tiled, K=64; the
                            # two heads run concurrently in the PE array)
                            nc.tensor.matmul(
                                s2[:, h2 * CHUNK + rel:(h2 + 1) * CHUNK],
                                lhsT=kt_sb[hp][h2 * DH:(h2 + 1) * DH,
                                               jt * P:(jt + 1) * P],
                                rhs=qt_sb[hp][h2 * DH:(h2 + 1) * DH,
                                              c0 + rel:c0 + CHUNK],
                                start=True, stop=not diag,
                                tile_position=(h2 * DH, 0))
                        if diag:
                            # additive causal mask accumulated into the S
                            # psum: identity.T @ maskneg tile (PE-local, no
                            # DVE in the S->exp->PV chain); one matmul per
                            # head — a single strided-AP matmul would cross
                            # the psum bank boundary between the two heads
                            for h2 in range(2):
                                nc.tensor.matmul(
                                    s2[:, h2 * CHUNK + rel:
                                       h2 * CHUNK + rel + P],
                                    lhsT=mask_sb[:, 2 * P:3 * P],
                                    rhs=mask_sb[:, h2 * P:(h2 + 1) * P],
                                    start=False, stop=True)
                        # one exp for both heads (both psum banks); flat AP
                        # off the diagonal (3D APs cost ~190ns extra on ACT)
                        if rel == 0:
                            nc.scalar.activation(es[:], s2[:], ActFn.Exp)
                        else:
                            nc.scalar.activation(
                                es.rearrange("p (t c) -> p t c",
                                             t=2)[:, :, rel:],
                                s2.rearrange("p (t c) -> p t c",
                                             t=2)[:, :, rel:],
                                ActFn.Exp)
                        # software pipeline: independent queued matmuls keep
                        # the PE streaming while exp(t) runs on ACT, then the
                        # previous slot's PV issues
                        # early chunks carry their own QKV supply in few
                        # slots: a deeper pop budget burns the backlog under
                        # the exps instead of in pass-boundary bursts
                        step_work(3 if ci < 2 else 2, pos,
                                  slack=2.5 / (2 * jt_end))
                        flush()
                        pend[0] = (hp, jt, es, rel)
                flush()

                # the chunk's partial projection needs only this chunk's
                # normalized ot_sb slabs — eligible shortly into the next
                # chunk, no communication involved
                for m in range(nm):
                    work_late.append([ci + 1.05, INF, g_proj(ci, m)])

            # final drain: leftover QKV steps, then the last chunk's
            # projections (~5us of PE work)
            while work:
                for _ in work[0][1]:
                    pass
                work.popleft()
            while work_late:
                for _ in work_late[0][2]:
                    pass
                work_late.popleft()

    if compile:
        nc.compile()
    return nc


def make_in_maps(x, Wq, Wk, Wv, Wo, n_cores=N_CORES):
    import ml_dtypes
    bf16 = ml_dtypes.bfloat16
    scale = np.float32(DH ** -0.5)
    # additive band mask for the diagonal j-tile of S^T [j,i]: 0 where
    # j <= i, -30000 where j > i (duplicated side by side for both heads),
    # plus the identity used to matmul-accumulate it into the S psum
    mask_b = np.where(np.triu(np.ones((P, P), np.float32)) > 0,
                      0.0, MASKNEG).astype(np.float32)
    ident = np.eye(P, dtype=np.float32)
    mask2 = np.concatenate([mask_b, mask_b, ident], axis=1).astype(bf16)

    def pack(sl):
        # [ntk*128, W] -> [128, ntk*W]: row p holds k-tile blocks side by
        # side so the whole weight is one contiguous-row DMA
        ntk = sl.shape[0] // P
        w = sl.shape[1]
        return np.ascontiguousarray(
            sl.reshape(ntk, P, w).transpose(1, 0, 2).reshape(P, ntk * w)
        ).astype(bf16)

    def pack_pm(sl):
        # pair-major: [nk*128, 2*128] -> [128, 2*nk*128] with
        # out[p, (pair*nk + k)*128 + j] = sl[k*128 + p, pair*128 + j],
        # so each head-pair's weights are one contiguous half
        ntk = sl.shape[0] // P
        return np.ascontiguousarray(
            sl.reshape(ntk, P, 2, P).transpose(1, 2, 0, 3).reshape(
                P, 2 * ntk * P)
        ).astype(bf16)

    in_maps = []
    for c in range(n_cores):
        b, r = divmod(c, 4)
        hs = r * HS
        in_maps.append({
            "xT": np.ascontiguousarray(x[b].T).astype(bf16),
            "wq": pack_pm(Wq[:, hs:hs + HS] * scale),
            "wk": pack_pm(Wk[:, hs:hs + HS]),
            "wv": pack(Wv[:, hs:hs + HS]),
            # row slice of Wo: this core's 256 inner rows, all columns
            "wo": pack(Wo[hs:hs + HS, :]),
            "mask_c": mask2,
        })
    return in_maps


def assemble_out(results, B, seq, n_cores=N_CORES):
    # sum the four per-core partials of each batch group in fp32
    out = np.zeros((B, seq, OUTD), np.float32)
    for c in range(n_cores):
        b, r = divmod(c, 4)
        out[b] += results[c]["outT"].T.astype(np.float32)
    return out


_NC_CACHE = {}


def kernel(x, Wq, Wk, Wv, Wo):
    from concourse import bass_utils
    x = np.asarray(x, np.float32)
    B, seq, dim = x.shape
    key = (seq, dim)
    if key not in _NC_CACHE:
        _NC_CACHE[key] = build_nc(seq=seq, dim=dim)
    nc = _NC_CACHE[key]
    in_maps = make_in_maps(x, np.asarray(Wq, np.float32),
                           np.asarray(Wk, np.float32),
                           np.asarray(Wv, np.float32),
                           np.asarray(Wo, np.float32))
    res = bass_utils.run_bass_kernel_spmd(
        nc, in_maps, core_ids=list(range(N_CORES)))
    return assemble_out(res.results, B, seq)
